# revision 53
# baseline (speedup 1.0000x reference)
"""Trainium2 Bass kernel for nn_MultiHeadAttention_78237124264578.

Reference computation (NO softmax; attention is purely bilinear):
    q = (x @ Wq.T + bq).reshape(8, 2, 2048, 64)   # FLAT reshape
    att = einsum('hbid,hbjd->hbij', q, k) * 64**-0.5
    out = einsum('hbij,hbjd->hbid', att, v)
    return out.transpose(1,2,3,0).reshape(2, 2048, 512)

Identities exploited (same as the bf16 baseline):
  1. (q kT) v == q (kT v): the attention matrix collapses to a 64x64
     Gram matrix S = K^T V per (head, 256-row block b2).
  2. The head reshape is flat: head h of Q/K/V is rows [512h, 512h+512)
     of the [4096, 512] projection output, so core i only needs x rows
     [512i, 512(i+1)) plus the full 512x512 weight matrices.
  3. O^T[f', r] per 128-row chunk is one matmul with the block-diagonal
     [S; S] as the stationary operand.

Speed trick on top: fp8e4 DoubleRow matmuls (2 contraction tiles per
instruction at 0.5 cycles/row -> 4x bf16 throughput in the cost model).
Full fp8 is too lossy (6.6% rel err), so every projection is computed
as a 3-term compensated product

    Y*256 = xh@Wh + xh@Wl + xl@Wh,   xh=fp8(x),     xl=fp8(x-xh)
                                     Wh=fp8(256 W), Wl=fp8(256 W - Wh)

which lands at ~0.4% overall rel err (bf16-comparable) at 0.75x the
bf16 PE cycle count (18432 -> plus small S/O stages in bf16).
Weights are scaled by 256 so fp8 normals cover them; the scale is
unwound on the host (output / 2^24) since S and O inherit 256^2 and
256^3 factors.  K's bias (free-dim, so neither ACT-bias nor
tensor_scalar can apply it) rides INSIDE the psum accumulation as a
rank-1 DoubleRow term outer(ones, bias_hi+bias_lo) -- those 8 extra
matmuls are free because they fill the wait for the wkl/xl DMAs.  V's
bias is a DVE tensor_add on the PSUM drain instead (zero extra engine
time vs a plain copy); K drains split across ACT/DVE (Pool cannot read
PSUM).  Q bias varies along partitions and uses ACT activation-bias /
DVE tensor_scalar_add, with the last bank split in halves across both
engines so the final output chunk drains fastest.

Input is packed into one [128, 32, 512] fp8 dram tensor in consumption
order, fetched with eight 4-slot DMAs on the SP/HWDGE queue; the three
small bias tensors ride the Pool/SWDGE queue so they never contend for
HWDGE.  Output ships as four [128, 512] bf16 DMAs as O chunks drain.
"""

import functools

import numpy as np

NCORES = 8
NIN = 512
NF = 512
R = 512
DIM = 64
SCALE = DIM ** -0.5
WS = 256.0           # weight scale so fp8e4 sees normal-range values
OUT_DESCALE = 1.0 / (WS * WS * WS)

N_WARMUP = 5

# pair-base slots in the packed input (pair i = slots [b, b+1])
XH = (0, 4)      # xh chunks (c0,c1) and (c2,c3)
WKH = (2, 6)
WKL = (8, 10)
XL = (12, 14)
WVH = (16, 18)
WVL = (20, 22)
WQH = (24, 26)
WQL = (28, 30)


@functools.lru_cache(maxsize=1)
def _build():
    from concourse import bacc
    import concourse.mybir as mybir
    import concourse.tile as tile

    f32 = mybir.dt.float32
    bf16 = mybir.dt.bfloat16
    f8 = mybir.dt.float8e4
    DR = mybir.MatmulPerfMode.DoubleRow
    IDT = mybir.ActivationFunctionType.Identity

    nc = bacc.Bacc(None, target_bir_lowering=False)

    inp_d = nc.dram_tensor("inp", [128, 32, 512], f8, kind="ExternalInput")
    # bias8 rows: 0 ones, 1 ones, 2 bkh, 3 bkl, 4 bvh, 5 bvl  (fp8, x256)
    bias8_d = nc.dram_tensor("bias8", [1, 6, 512], f8, kind="ExternalInput")
    brow_d = nc.dram_tensor("brow", [1, NF], f32, kind="ExternalInput")  # 256*bv
    bqc_d = nc.dram_tensor("bqc", [128, 4], f32, kind="ExternalInput")  # 256*SCALE*bq
    ot_d = nc.dram_tensor("ot", [NF, R], bf16, kind="ExternalOutput")

    with tile.TileContext(nc) as tc:
        with (
            tc.tile_pool(name="sb", bufs=1) as sb,
            tc.tile_pool(name="pa", bufs=4, space="PSUM") as pa,
            tc.tile_pool(name="pb", bufs=4, space="PSUM") as pb,
        ):
            # ---- PE warm-up: start the p-state ramp early -----------------
            wu = sb.tile([1, 128], f32, tag="wu", name="wu")
            nc.gpsimd.memset(wu[:], 0.0)
            for i in range(N_WARMUP):
                psw = pb.tile([1, 128], f32, tag="B", name=f"psw{i}")
                nc.tensor.matmul(psw[:], wu[0:1, 0:1], wu[:])

            # ---- DMAs ------------------------------------------------------
            ops = sb.tile([128, 32, 512], f8, tag="ops", name="ops")
            bias8 = sb.tile([1, 6, 512], f8, tag="bias8")
            bqc = sb.tile([128, 4], f32, tag="bqc")
            brow = sb.tile([1, NF], f32, tag="brow")
            nc.gpsimd.dma_start(bias8[:], bias8_d[:, :, :])
            nc.gpsimd.dma_start(bqc[:], bqc_d[:, :])
            nc.gpsimd.dma_start(brow[:], brow_d[:, :])
            bvb = sb.tile([128, NF], f32, tag="bvb")
            nc.gpsimd.partition_broadcast(bvb[:], brow[0:1, :])
            for t in range(8):
                nc.sync.dma_start(
                    ops[:, 4 * t:4 * t + 4, :], inp_d[:, 4 * t:4 * t + 4, :])

            s2b = [sb.tile([128, 128], bf16, tag=f"s2b{b}", name=f"s2b{b}")
                   for b in range(2)]
            nc.gpsimd.memset(s2b[0][:], 0.0)
            nc.gpsimd.memset(s2b[1][:], 0.0)

            k_sb = [sb.tile([128, NF], bf16, tag=f"k{c}", name=f"k{c}") for c in range(4)]
            v_sb = [sb.tile([128, NF], bf16, tag=f"v{c}", name=f"v{c}") for c in range(4)]
            q_sb = [sb.tile([128, R], bf16, tag=f"q{c}", name=f"q{c}") for c in range(3)]
            # last Q chunk / O chunk as separate half tiles: disjoint slices
            # of one tile serialize in the dependency tracker
            q3h = [sb.tile([128, 256], bf16, tag=f"q3h{b}", name=f"q3h{b}")
                   for b in range(2)]
            oc_sb = [sb.tile([128, R], bf16, tag=f"oc{c}", name=f"oc{c}")
                     for c in range(4)]

            def pair(base, cols):
                return ops[:, base:base + 2, cols]

            # K/V: out[row-chunk rc, f]; stationary x pair, moving w pair
            def dr_kv(ps, rc, fh, xs, ws, start=False, stop=False):
                nc.tensor.matmul(
                    ps[:, 256 * fh:256 * fh + 256],
                    pair(xs, slice(128 * rc, 128 * rc + 128)),
                    pair(ws, slice(256 * fh, 256 * fh + 256)),
                    start=start, stop=stop, perf_mode=DR,
                )

            # Q: out[f-chunk c, r]; stationary w pair, moving x pair
            def dr_q(ps, c, rh, ws, xs, start=False, stop=False):
                nc.tensor.matmul(
                    ps[:, 256 * rh:256 * rh + 256],
                    pair(ws, slice(128 * c, 128 * c + 128)),
                    pair(xs, slice(256 * rh, 256 * rh + 256)),
                    start=start, stop=stop, perf_mode=DR,
                )

            # rank-1 bias term: outer(ones, bias_hi) + outer(ones, bias_lo)
            def dr_bias(ps, fh, brow, start=False):
                nc.tensor.matmul(
                    ps[:, 256 * fh:256 * fh + 256],
                    bias8[0:1, 0:2, 0:128],
                    bias8[0:1, brow:brow + 2, 256 * fh:256 * fh + 256],
                    start=start, stop=False, perf_mode=DR,
                )

            # ---- K projection ---------------------------------------------
            psk = [pa.tile([128, NF], f32, tag="A", name=f"psk{c}") for c in range(4)]
            for rc in range(4):                      # t1 p0 [needs D1]
                for fh in range(2):
                    dr_kv(psk[rc], rc, fh, XH[0], WKH[0], start=(fh == 0))
            for rc in range(3):                      # bias rows [tiny DMA]
                for fh in range(2):
                    dr_bias(psk[rc], fh, 2)
            psv = [pb.tile([128, NF], f32, tag="B", name=f"psv{c}") for c in range(4)]
            for rc in range(4):                      # t1 p1 [D2]
                for fh in range(2):
                    dr_kv(psk[rc], rc, fh, XH[1], WKH[1])
            for fh in range(2):                      # last bias pair covers
                dr_bias(psk[3], fh, 2)               # the wait for D3
            for p in range(2):                       # t2 [D3]
                for rc in range(4):
                    for fh in range(2):
                        dr_kv(psk[rc], rc, fh, XH[p], WKL[p])
            for rc in range(4):                      # t3 bank-major [D4]
                for p in range(2):
                    for fh in range(2):
                        dr_kv(psk[rc], rc, fh, XL[p], WKH[p],
                              stop=(p == 1 and fh == 1))
                if rc % 2 == 0:
                    nc.scalar.copy(k_sb[rc][:], psk[rc][:])
                else:
                    nc.vector.tensor_copy(k_sb[rc][:], psk[rc][:])

            # ---- V projection (bias already accumulated) ------------------
            for rc in range(4):                      # t1 p0/p1 [D5]
                for fh in range(2):
                    dr_kv(psv[rc], rc, fh, XH[0], WVH[0], start=(fh == 0))
            for rc in range(4):
                for fh in range(2):
                    dr_kv(psv[rc], rc, fh, XH[1], WVH[1])
            for rc in range(4):                      # t3 + t2 bank-major [D6]
                for p in range(2):
                    for fh in range(2):
                        dr_kv(psv[rc], rc, fh, XL[p], WVH[p])
                for p in range(2):
                    for fh in range(2):
                        dr_kv(psv[rc], rc, fh, XH[p], WVL[p],
                              stop=(p == 1 and fh == 1))
                nc.vector.tensor_add(v_sb[rc][:], psv[rc][:], bvb[:])

            # ---- S = K^T V, parts interleaved with Q banks so the PE never
            # waits on a straggling v-copy ----------------------------------
            def s_part(ps_s, rc, start, stop):
                for fh8 in range(8):
                    nc.tensor.matmul(
                        ps_s[:],
                        k_sb[rc][:, 64 * fh8:64 * fh8 + 64],
                        v_sb[rc][:, 64 * fh8:64 * fh8 + 64],
                        start=(start and fh8 == 0), stop=(stop and fh8 == 7),
                    )

            ps_s0 = pb.tile([64, 64], f32, tag="B", name="ps_s0")
            ps_s1 = pb.tile([64, 64], f32, tag="B", name="ps_s1")
            psq = [pa.tile([128, R], f32, tag="A", name=f"psq{c}") for c in range(4)]
            ps_oc = [pb.tile([128, R], f32, tag="B", name=f"ps_oc{c}")
                     for c in range(4)]

            def q_bank(c):
                for p in range(2):
                    for rh in range(2):
                        dr_q(psq[c], c, rh, WQH[p], XH[p],
                             start=(p == 0 and rh == 0))
                for p in range(2):
                    for rh in range(2):
                        dr_q(psq[c], c, rh, WQH[p], XL[p])
                for p in range(2):
                    for rh in range(2):
                        dr_q(psq[c], c, rh, WQL[p], XH[p],
                             stop=(p == 1 and rh == 1))
                if c == 0:
                    nc.scalar.activation(
                        q_sb[0][:], psq[0][:], IDT,
                        bias=bqc[:, 0:1], scale=1.0)
                elif c == 1:
                    nc.vector.tensor_scalar_add(
                        q_sb[1][:], psq[1][:], bqc[:, 1:2])
                elif c == 2:
                    nc.vector.tensor_scalar_add(
                        q_sb[2][:], psq[2][:], bqc[:, 2:3])
                else:
                    # halves on both engines so the last bank drains fast
                    nc.scalar.activation(
                        q3h[0][:], psq[3][:, 0:256], IDT,
                        bias=bqc[:, 3:4], scale=1.0)
                    nc.vector.tensor_scalar_add(
                        q3h[1][:], psq[3][:, 256:512], bqc[:, 3:4])

            def o_mm(c, b2):
                rsl = slice(256 * b2, 256 * (b2 + 1))
                nc.tensor.matmul(ps_oc[c][:, rsl], s2b[b2][:], q_sb[c][:, rsl],
                                 start=(b2 == 0), stop=(b2 == 1))

            q_bank(0)
            s_part(ps_s0, 0, True, False)
            s_part(ps_s0, 1, False, True)
            nc.scalar.copy(s2b[0][0:64, 0:64], ps_s0[:])
            nc.scalar.copy(s2b[0][64:128, 64:128], ps_s0[:])
            q_bank(1)
            s_part(ps_s1, 2, True, False)
            s_part(ps_s1, 3, False, True)
            nc.vector.tensor_copy(s2b[1][0:64, 0:64], ps_s1[:])
            nc.vector.tensor_copy(s2b[1][64:128, 64:128], ps_s1[:])
            q_bank(2)
            o_mm(0, 0)
            o_mm(0, 1)
            nc.scalar.copy(oc_sb[0][:], ps_oc[0][:])
            nc.sync.dma_start(ot_d[0:128, :], oc_sb[0][:])
            o_mm(1, 0)
            o_mm(1, 1)
            nc.vector.tensor_copy(oc_sb[1][:], ps_oc[1][:])
            nc.sync.dma_start(ot_d[128:256, :], oc_sb[1][:])
            q_bank(3)
            o_mm(2, 0)
            o_mm(2, 1)
            nc.scalar.copy(oc_sb[2][:], ps_oc[2][:])
            nc.sync.dma_start(ot_d[256:384, :], oc_sb[2][:])
            nc.tensor.matmul(ps_oc[3][:, 0:256], s2b[0][:], q3h[0][:],
                             start=True, stop=False)
            nc.tensor.matmul(ps_oc[3][:, 256:512], s2b[1][:], q3h[1][:],
                             start=False, stop=True)
            # last chunk: qbias ran as parallel halves, but copy + ship as ONE
            # [128,512] DMA — a fifth HWDGE hold (625ns) costs more than the
            # larger copy
            nc.vector.tensor_copy(oc_sb[3][:], ps_oc[3][:])
            nc.sync.dma_start(ot_d[384:512, :], oc_sb[3][:])

    nc.compile()
    return nc


def kernel(x, Wq, bq, Wk, bk, Wv, bv):
    import ml_dtypes
    from concourse.bass_utils import run_bass_kernel_spmd

    f8 = ml_dtypes.float8_e4m3

    x = np.asarray(x, dtype=np.float32)
    Wq = np.asarray(Wq, dtype=np.float32)
    Wk = np.asarray(Wk, dtype=np.float32)
    Wv = np.asarray(Wv, dtype=np.float32)
    bq = np.asarray(bq, dtype=np.float32)
    bk = np.asarray(bk, dtype=np.float32)
    bv = np.asarray(bv, dtype=np.float32)

    B, N, nin = x.shape
    x_flat = x.reshape(B * N, nin)

    def split8(a):
        hi = np.asarray(a, f8)
        lo = np.asarray(a - hi.astype(np.float32), f8)
        return hi, lo

    def chunks(t):
        return [t[128 * j:128 * (j + 1)] for j in range(4)]

    wkh, wkl = split8(WS * Wk.T)
    wvh, wvl = split8(WS * Wv.T)
    wqh, wql = split8(WS * SCALE * Wq.T)
    wkh_c, wkl_c = chunks(wkh), chunks(wkl)
    wvh_c, wvl_c = chunks(wvh), chunks(wvl)
    wqh_c, wql_c = chunks(wqh), chunks(wql)

    bkh, bkl = split8(WS * bk)
    bvh, bvl = split8(WS * bv)
    brow = np.ascontiguousarray((WS * bv).reshape(1, NF).astype(np.float32))
    ones = np.ones(512, f8)
    bias8 = np.ascontiguousarray(
        np.stack([ones, ones, bkh, bkl, bvh, bvl])[None, :, :])
    bqc = np.ascontiguousarray(
        (WS * SCALE * bq).reshape(4, 128).T)              # [p, c] f32

    in_maps = []
    for i in range(NCORES):
        xt = x_flat[R * i:R * (i + 1)].T                  # [in, r]
        xh, xl = split8(xt)
        xh_c, xl_c = chunks(xh), chunks(xl)
        slots = [
            xh_c[0], xh_c[1], wkh_c[0], wkh_c[1],
            xh_c[2], xh_c[3], wkh_c[2], wkh_c[3],
            wkl_c[0], wkl_c[1], wkl_c[2], wkl_c[3],
            xl_c[0], xl_c[1], xl_c[2], xl_c[3],
            wvh_c[0], wvh_c[1], wvh_c[2], wvh_c[3],
            wvl_c[0], wvl_c[1], wvl_c[2], wvl_c[3],
            wqh_c[0], wqh_c[1], wqh_c[2], wqh_c[3],
            wql_c[0], wql_c[1], wql_c[2], wql_c[3],
        ]
        inp = np.ascontiguousarray(np.stack(slots, axis=1))  # [128, 32, 512]
        in_maps.append({"inp": inp, "bias8": bias8, "brow": brow, "bqc": bqc})

    nc = _build()
    res = run_bass_kernel_spmd(nc, in_maps, core_ids=list(range(NCORES)))

    # ot[i][fh*64+d, b2*256+rr] = 2^24 * out[h=i, b2, n2=rr*8+fh, d]
    ot = np.stack([np.asarray(res.results[i]["ot"], dtype=np.float32)
                   for i in range(NCORES)])                       # [h, f', r]
    ot *= OUT_DESCALE
    ot = ot.reshape(NCORES, 8, DIM, 2, 256)                       # [h, fh, d, b2, rr]
    z = ot.transpose(3, 4, 1, 2, 0).reshape(B, N, 8 * DIM)        # [b2, n2, d*8+h]
    return np.ascontiguousarray(z)


# revision 54
# speedup vs baseline: 1.0011x; 1.0011x over previous
"""Trainium2 Bass kernel for nn_MultiHeadAttention_78237124264578.

Reference computation (NO softmax; attention is purely bilinear):
    q = (x @ Wq.T + bq).reshape(8, 2, 2048, 64)   # FLAT reshape
    att = einsum('hbid,hbjd->hbij', q, k) * 64**-0.5
    out = einsum('hbij,hbjd->hbid', att, v)
    return out.transpose(1,2,3,0).reshape(2, 2048, 512)

Identities exploited (same as the bf16 baseline):
  1. (q kT) v == q (kT v): the attention matrix collapses to a 64x64
     Gram matrix S = K^T V per (head, 256-row block b2).
  2. The head reshape is flat: head h of Q/K/V is rows [512h, 512h+512)
     of the [4096, 512] projection output, so core i only needs x rows
     [512i, 512(i+1)) plus the full 512x512 weight matrices.
  3. O^T[f', r] per 128-row chunk is one matmul with the block-diagonal
     [S; S] as the stationary operand.

Speed trick on top: fp8e4 DoubleRow matmuls (2 contraction tiles per
instruction at 0.5 cycles/row -> 4x bf16 throughput in the cost model).
Full fp8 is too lossy (6.6% rel err), so every projection is computed
as a 3-term compensated product

    Y*256 = xh@Wh + xh@Wl + xl@Wh,   xh=fp8(x),     xl=fp8(x-xh)
                                     Wh=fp8(256 W), Wl=fp8(256 W - Wh)

which lands at ~0.4% overall rel err (bf16-comparable) at 0.75x the
bf16 PE cycle count (18432 -> plus small S/O stages in bf16).
Weights are scaled by 256 so fp8 normals cover them; the scale is
unwound on the host (output / 2^24) since S and O inherit 256^2 and
256^3 factors.  K's bias (free-dim, so neither ACT-bias nor
tensor_scalar can apply it) rides INSIDE the psum accumulation as a
rank-1 DoubleRow term outer(ones, bias_hi+bias_lo) -- those 8 extra
matmuls are free because they fill the wait for the wkl/xl DMAs.  V's
bias is a DVE tensor_add on the PSUM drain instead (zero extra engine
time vs a plain copy); K drains split across ACT/DVE (Pool cannot read
PSUM).  Q bias varies along partitions and uses ACT activation-bias /
DVE tensor_scalar_add, with the last bank split in halves across both
engines so the final output chunk drains fastest.

Input is packed into one [128, 32, 512] fp8 dram tensor in consumption
order, fetched with eight 4-slot DMAs on the SP/HWDGE queue; the three
small bias tensors ride the Pool/SWDGE queue so they never contend for
HWDGE.  Output ships as four [128, 512] bf16 DMAs as O chunks drain.
"""

import functools

import numpy as np

NCORES = 8
NIN = 512
NF = 512
R = 512
DIM = 64
SCALE = DIM ** -0.5
WS = 256.0           # weight scale so fp8e4 sees normal-range values
OUT_DESCALE = 1.0 / (WS * WS * WS)

N_WARMUP = 5

# pair-base slots in the packed input (pair i = slots [b, b+1])
XH = (0, 4)      # xh chunks (c0,c1) and (c2,c3)
WKH = (2, 6)
WKL = (8, 10)
XL = (12, 14)
WVH = (16, 18)
WVL = (20, 22)
WQH = (24, 26)
WQL = (28, 30)


@functools.lru_cache(maxsize=1)
def _build():
    from concourse import bacc
    import concourse.mybir as mybir
    import concourse.tile as tile

    f32 = mybir.dt.float32
    bf16 = mybir.dt.bfloat16
    f8 = mybir.dt.float8e4
    DR = mybir.MatmulPerfMode.DoubleRow
    IDT = mybir.ActivationFunctionType.Identity

    nc = bacc.Bacc(None, target_bir_lowering=False)

    inp_d = nc.dram_tensor("inp", [128, 32, 512], f8, kind="ExternalInput")
    # bias8 rows: 0 ones, 1 ones, 2 bkh, 3 bkl, 4 bvh, 5 bvl  (fp8, x256)
    bias8_d = nc.dram_tensor("bias8", [1, 6, 512], f8, kind="ExternalInput")
    brow_d = nc.dram_tensor("brow", [1, NF], f32, kind="ExternalInput")  # 256*bv
    bqc_d = nc.dram_tensor("bqc", [128, 4], f32, kind="ExternalInput")  # 256*SCALE*bq
    ot_d = nc.dram_tensor("ot", [NF, R], bf16, kind="ExternalOutput")

    with tile.TileContext(nc) as tc:
        with (
            tc.tile_pool(name="sb", bufs=1) as sb,
            tc.tile_pool(name="pa", bufs=4, space="PSUM") as pa,
            tc.tile_pool(name="pb", bufs=4, space="PSUM") as pb,
        ):
            # ---- PE warm-up: start the p-state ramp early -----------------
            wu = sb.tile([1, 128], f32, tag="wu", name="wu")
            nc.gpsimd.memset(wu[:], 0.0)
            for i in range(N_WARMUP):
                psw = pb.tile([1, 128], f32, tag="B", name=f"psw{i}")
                nc.tensor.matmul(psw[:], wu[0:1, 0:1], wu[:])

            # ---- DMAs ------------------------------------------------------
            ops = sb.tile([128, 32, 512], f8, tag="ops", name="ops")
            bias8 = sb.tile([1, 6, 512], f8, tag="bias8")
            bqc = sb.tile([128, 4], f32, tag="bqc")
            brow = sb.tile([1, NF], f32, tag="brow")
            nc.gpsimd.dma_start(bias8[:], bias8_d[:, :, :])
            nc.gpsimd.dma_start(bqc[:], bqc_d[:, :])
            nc.gpsimd.dma_start(brow[:], brow_d[:, :])
            bvb = sb.tile([128, NF], f32, tag="bvb")
            nc.gpsimd.partition_broadcast(bvb[:], brow[0:1, :])
            for t in range(8):
                nc.sync.dma_start(
                    ops[:, 4 * t:4 * t + 4, :], inp_d[:, 4 * t:4 * t + 4, :])

            s2b = [sb.tile([128, 128], bf16, tag=f"s2b{b}", name=f"s2b{b}")
                   for b in range(2)]
            nc.gpsimd.memset(s2b[0][:], 0.0)
            nc.gpsimd.memset(s2b[1][:], 0.0)

            k_sb = [sb.tile([128, NF], bf16, tag=f"k{c}", name=f"k{c}") for c in range(4)]
            v_sb = [sb.tile([128, NF], bf16, tag=f"v{c}", name=f"v{c}") for c in range(4)]
            q_sb = [sb.tile([128, R], bf16, tag=f"q{c}", name=f"q{c}") for c in range(3)]
            # last Q chunk / O chunk as separate half tiles: disjoint slices
            # of one tile serialize in the dependency tracker
            q3h = [sb.tile([128, 256], bf16, tag=f"q3h{b}", name=f"q3h{b}")
                   for b in range(2)]
            oc_sb = [sb.tile([128, R], bf16, tag=f"oc{c}", name=f"oc{c}")
                     for c in range(4)]

            def pair(base, cols):
                return ops[:, base:base + 2, cols]

            # K/V: out[row-chunk rc, f]; stationary x pair, moving w pair
            def dr_kv(ps, rc, fh, xs, ws, start=False, stop=False):
                nc.tensor.matmul(
                    ps[:, 256 * fh:256 * fh + 256],
                    pair(xs, slice(128 * rc, 128 * rc + 128)),
                    pair(ws, slice(256 * fh, 256 * fh + 256)),
                    start=start, stop=stop, perf_mode=DR,
                )

            # Q: out[f-chunk c, r]; stationary w pair, moving x pair
            def dr_q(ps, c, rh, ws, xs, start=False, stop=False):
                nc.tensor.matmul(
                    ps[:, 256 * rh:256 * rh + 256],
                    pair(ws, slice(128 * c, 128 * c + 128)),
                    pair(xs, slice(256 * rh, 256 * rh + 256)),
                    start=start, stop=stop, perf_mode=DR,
                )

            # rank-1 bias term: outer(ones, bias_hi) + outer(ones, bias_lo)
            def dr_bias(ps, fh, brow, start=False):
                nc.tensor.matmul(
                    ps[:, 256 * fh:256 * fh + 256],
                    bias8[0:1, 0:2, 0:128],
                    bias8[0:1, brow:brow + 2, 256 * fh:256 * fh + 256],
                    start=start, stop=False, perf_mode=DR,
                )

            # ---- K projection ---------------------------------------------
            psk = [pa.tile([128, NF], f32, tag="A", name=f"psk{c}") for c in range(4)]
            for rc in range(4):                      # t1 p0 [needs D1]
                for fh in range(2):
                    dr_kv(psk[rc], rc, fh, XH[0], WKH[0], start=(fh == 0))
            for rc in range(3):                      # bias rows [tiny DMA]
                for fh in range(2):
                    dr_bias(psk[rc], fh, 2)
            psv = [pb.tile([128, NF], f32, tag="B", name=f"psv{c}") for c in range(4)]
            for rc in range(4):                      # t1 p1 [D2]
                for fh in range(2):
                    dr_kv(psk[rc], rc, fh, XH[1], WKH[1])
            for fh in range(2):                      # last bias pair covers
                dr_bias(psk[3], fh, 2)               # the wait for D3
            for p in range(2):                       # t2 [D3]
                for rc in range(4):
                    for fh in range(2):
                        dr_kv(psk[rc], rc, fh, XH[p], WKL[p])
            for rc in range(4):                      # t3 bank-major [D4]
                for p in range(2):
                    for fh in range(2):
                        dr_kv(psk[rc], rc, fh, XL[p], WKH[p],
                              stop=(p == 1 and fh == 1))
                if rc % 2 == 0:
                    nc.scalar.copy(k_sb[rc][:], psk[rc][:])
                else:
                    nc.vector.tensor_copy(k_sb[rc][:], psk[rc][:])

            # ---- V projection (bias already accumulated) ------------------
            for rc in range(4):                      # t1 p0/p1 [D5]
                for fh in range(2):
                    dr_kv(psv[rc], rc, fh, XH[0], WVH[0], start=(fh == 0))
            for rc in range(4):
                for fh in range(2):
                    dr_kv(psv[rc], rc, fh, XH[1], WVH[1])
            for rc in range(4):                      # t3 + t2 bank-major [D6]
                for p in range(2):
                    for fh in range(2):
                        dr_kv(psv[rc], rc, fh, XL[p], WVH[p])
                for p in range(2):
                    for fh in range(2):
                        dr_kv(psv[rc], rc, fh, XH[p], WVL[p],
                              stop=(p == 1 and fh == 1))
                nc.vector.tensor_add(v_sb[rc][:], psv[rc][:], bvb[:])

            # ---- S = K^T V, parts interleaved with Q banks so the PE never
            # waits on a straggling v-copy ----------------------------------
            def s_part(ps_s, rc, start, stop):
                for fh8 in range(8):
                    nc.tensor.matmul(
                        ps_s[:],
                        k_sb[rc][:, 64 * fh8:64 * fh8 + 64],
                        v_sb[rc][:, 64 * fh8:64 * fh8 + 64],
                        start=(start and fh8 == 0), stop=(stop and fh8 == 7),
                    )

            ps_s0 = pb.tile([64, 64], f32, tag="B", name="ps_s0")
            ps_s1 = pb.tile([64, 64], f32, tag="B", name="ps_s1")
            psq = [pa.tile([128, R], f32, tag="A", name=f"psq{c}") for c in range(4)]
            ps_oc = [pb.tile([128, R], f32, tag="B", name=f"ps_oc{c}")
                     for c in range(4)]

            def q_bank(c):
                for p in range(2):
                    for rh in range(2):
                        dr_q(psq[c], c, rh, WQH[p], XH[p],
                             start=(p == 0 and rh == 0))
                for p in range(2):
                    for rh in range(2):
                        dr_q(psq[c], c, rh, WQH[p], XL[p])
                for p in range(2):
                    for rh in range(2):
                        dr_q(psq[c], c, rh, WQL[p], XH[p],
                             stop=(p == 1 and rh == 1))
                if c == 0:
                    nc.scalar.activation(
                        q_sb[0][:], psq[0][:], IDT,
                        bias=bqc[:, 0:1], scale=1.0)
                elif c == 1:
                    nc.vector.tensor_scalar_add(
                        q_sb[1][:], psq[1][:], bqc[:, 1:2])
                elif c == 2:
                    nc.vector.tensor_scalar_add(
                        q_sb[2][:], psq[2][:], bqc[:, 2:3])
                else:
                    # halves on both engines so the last bank drains fast
                    nc.scalar.activation(
                        q3h[0][:], psq[3][:, 0:256], IDT,
                        bias=bqc[:, 3:4], scale=1.0)
                    nc.vector.tensor_scalar_add(
                        q3h[1][:], psq[3][:, 256:512], bqc[:, 3:4])

            def o_mm(c, b2):
                rsl = slice(256 * b2, 256 * (b2 + 1))
                nc.tensor.matmul(ps_oc[c][:, rsl], s2b[b2][:], q_sb[c][:, rsl],
                                 start=(b2 == 0), stop=(b2 == 1))

            q_bank(0)
            s_part(ps_s0, 0, True, False)
            s_part(ps_s0, 1, False, True)
            nc.scalar.copy(s2b[0][0:64, 0:64], ps_s0[:])
            nc.scalar.copy(s2b[0][64:128, 64:128], ps_s0[:])
            q_bank(1)
            s_part(ps_s1, 2, True, False)
            s_part(ps_s1, 3, False, True)
            nc.vector.tensor_copy(s2b[1][0:64, 0:64], ps_s1[:])
            nc.vector.tensor_copy(s2b[1][64:128, 64:128], ps_s1[:])
            q_bank(2)
            o_mm(0, 0)
            o_mm(0, 1)
            nc.scalar.copy(oc_sb[0][:], ps_oc[0][:])
            nc.sync.dma_start(ot_d[0:128, :], oc_sb[0][:])
            o_mm(1, 0)
            o_mm(1, 1)
            nc.vector.tensor_copy(oc_sb[1][:], ps_oc[1][:])
            q_bank(3)
            o_mm(2, 0)
            o_mm(2, 1)
            nc.scalar.copy(oc_sb[2][:], ps_oc[2][:])
            nc.sync.dma_start(ot_d[256:384, :], oc_sb[2][:])
            # c1 ships via the ACT queue: its wait is already satisfied when
            # ACT.SEQ reaches it, and its HWDGE hold overlaps SP's issues
            nc.scalar.dma_start(ot_d[128:256, :], oc_sb[1][:])
            nc.tensor.matmul(ps_oc[3][:, 0:256], s2b[0][:], q3h[0][:],
                             start=True, stop=False)
            nc.tensor.matmul(ps_oc[3][:, 256:512], s2b[1][:], q3h[1][:],
                             start=False, stop=True)
            # last chunk: qbias ran as parallel halves, but copy + ship as ONE
            # [128,512] DMA — a fifth HWDGE hold (625ns) costs more than the
            # larger copy
            nc.vector.tensor_copy(oc_sb[3][:], ps_oc[3][:])
            nc.sync.dma_start(ot_d[384:512, :], oc_sb[3][:])

    nc.compile()
    return nc


def kernel(x, Wq, bq, Wk, bk, Wv, bv):
    import ml_dtypes
    from concourse.bass_utils import run_bass_kernel_spmd

    f8 = ml_dtypes.float8_e4m3

    x = np.asarray(x, dtype=np.float32)
    Wq = np.asarray(Wq, dtype=np.float32)
    Wk = np.asarray(Wk, dtype=np.float32)
    Wv = np.asarray(Wv, dtype=np.float32)
    bq = np.asarray(bq, dtype=np.float32)
    bk = np.asarray(bk, dtype=np.float32)
    bv = np.asarray(bv, dtype=np.float32)

    B, N, nin = x.shape
    x_flat = x.reshape(B * N, nin)

    def split8(a):
        hi = np.asarray(a, f8)
        lo = np.asarray(a - hi.astype(np.float32), f8)
        return hi, lo

    def chunks(t):
        return [t[128 * j:128 * (j + 1)] for j in range(4)]

    wkh, wkl = split8(WS * Wk.T)
    wvh, wvl = split8(WS * Wv.T)
    wqh, wql = split8(WS * SCALE * Wq.T)
    wkh_c, wkl_c = chunks(wkh), chunks(wkl)
    wvh_c, wvl_c = chunks(wvh), chunks(wvl)
    wqh_c, wql_c = chunks(wqh), chunks(wql)

    bkh, bkl = split8(WS * bk)
    bvh, bvl = split8(WS * bv)
    brow = np.ascontiguousarray((WS * bv).reshape(1, NF).astype(np.float32))
    ones = np.ones(512, f8)
    bias8 = np.ascontiguousarray(
        np.stack([ones, ones, bkh, bkl, bvh, bvl])[None, :, :])
    bqc = np.ascontiguousarray(
        (WS * SCALE * bq).reshape(4, 128).T)              # [p, c] f32

    in_maps = []
    for i in range(NCORES):
        xt = x_flat[R * i:R * (i + 1)].T                  # [in, r]
        xh, xl = split8(xt)
        xh_c, xl_c = chunks(xh), chunks(xl)
        slots = [
            xh_c[0], xh_c[1], wkh_c[0], wkh_c[1],
            xh_c[2], xh_c[3], wkh_c[2], wkh_c[3],
            wkl_c[0], wkl_c[1], wkl_c[2], wkl_c[3],
            xl_c[0], xl_c[1], xl_c[2], xl_c[3],
            wvh_c[0], wvh_c[1], wvh_c[2], wvh_c[3],
            wvl_c[0], wvl_c[1], wvl_c[2], wvl_c[3],
            wqh_c[0], wqh_c[1], wqh_c[2], wqh_c[3],
            wql_c[0], wql_c[1], wql_c[2], wql_c[3],
        ]
        inp = np.ascontiguousarray(np.stack(slots, axis=1))  # [128, 32, 512]
        in_maps.append({"inp": inp, "bias8": bias8, "brow": brow, "bqc": bqc})

    nc = _build()
    res = run_bass_kernel_spmd(nc, in_maps, core_ids=list(range(NCORES)))

    # ot[i][fh*64+d, b2*256+rr] = 2^24 * out[h=i, b2, n2=rr*8+fh, d]
    ot = np.stack([np.asarray(res.results[i]["ot"], dtype=np.float32)
                   for i in range(NCORES)])                       # [h, f', r]
    ot *= OUT_DESCALE
    ot = ot.reshape(NCORES, 8, DIM, 2, 256)                       # [h, fh, d, b2, rr]
    z = ot.transpose(3, 4, 1, 2, 0).reshape(B, N, 8 * DIM)        # [b2, n2, d*8+h]
    return np.ascontiguousarray(z)
